# revision 6
# baseline (speedup 1.0000x reference)
"""Criss-cross (CCNet) sparse attention kernel for Trainium2, 8-core data-parallel.

Problem (hardcoded): B=8, CQ=64, CV=512, H=W=128, fp32 I/O.
Per core: one image.  reference:
    energy_H[i,w,j] = sum_c q[c,i,w] k[c,j,w]   (diag i==j masked -inf)
    energy_W[i,w,j] = sum_c q[c,i,w] k[c,i,j]
    att = softmax(concat(energy_H, energy_W), axis=j)  (256-way per pixel)
    out[c,i,w] = sum_j v[c,j,w] att_H[i,w,j] + sum_j v[c,i,j] att_W[i,w,j]

Kernel strategy (v4) — baseline phase 1 + merged-psum PV:
  - q/k cast to fp16 on load; energies per row i / col w -> exp(E-40) ->
    att_W[j, i*W+w], att_H[j, w*H+i] bf16; att_H diagonal zeroed by (1-I)
    mask mult before the denominators; denominators via basis-matmul psum
    accumulation; f32 reciprocal; flat bf16 r2 slices feed rank-1
    broadcast matmuls; att *= 1/dn on DVE (ACT-staged prb for odd quads).
  - v chunk (128 ch) loaded as one c-major cast SWDGE DMA; PE transposes
    build u2[j, w, c] (col-pass lhsT, full chunk) and zg[j, i, c]
    (row-pass lhsT, per 8-i group, just-in-time, double-buffered).
  - PV: both passes accumulate into ONE psum group pg[c, 8i, 128w]
    (2 banks): rows d=0..7 first (start only on d=0 and d=4 - exactly one
    zero-region start per 2KB bank), then 128x2 col matmuls with 4-i
    strided within-bank writes, start=False: they replace pending-zero
    bytes / accumulate on already-written ones (PE zero-region
    semantics).  This kills the baseline's separate col-psum ACT copy
    and row-psum DVE add.
  - single copy per group psum -> out_sb (ACT mostly; transpose evacs
    lean DVE where bf16 runs 2x), then one 4KB-run store DMA per group.
"""

import threading

import numpy as np

CQ, CV, H, W = 64, 512, 128, 128
PIX = H * W
B = 8
EXP_BIAS = -40.0
CHUNK = 128
N_CHUNKS = CV // CHUNK
GI = 8                      # i-rows per merged psum group (2 psum banks)
NG = H // GI                # groups per chunk


def build_nc():
    import concourse.mybir as mybir
    import concourse.tile as tile
    from concourse import bacc
    from concourse.masks import make_identity

    f32 = mybir.dt.float32
    bf16 = mybir.dt.bfloat16
    fp16 = mybir.dt.float16
    Exp = mybir.ActivationFunctionType.Exp
    add = mybir.AluOpType.add
    mult = mybir.AluOpType.mult

    nc = bacc.Bacc(None, target_bir_lowering=False)

    with tile.TileContext(nc) as tc:
        with (
            tc.tile_pool(name="dram", bufs=1, space="DRAM") as dram,
            tc.tile_pool(name="attp", bufs=1) as attp,
            tc.tile_pool(name="constp", bufs=1) as constp,
            tc.tile_pool(name="dnp", bufs=1) as dnp,
            tc.tile_pool(name="vp", bufs=2) as vp,
        ):
            q_d = dram.tile((CQ, H, W), f32, kind="ExternalInput", name="q", uniquify=False)
            k_d = dram.tile((CQ, H, W), f32, kind="ExternalInput", name="k", uniquify=False)
            v_d = dram.tile((CV, H, W), f32, kind="ExternalInput", name="v", uniquify=False)
            o_d = dram.tile((CV, H, W), f32, kind="ExternalOutput", name="o", uniquify=False)

            # att_W[j, i*W + w] ; att_H[j, w*H + i]  (bf16, denominator-scaled)
            att_W = attp.tile([128, PIX], bf16)
            att_H = attp.tile([128, PIX], bf16)

            ident = dnp.tile([128, 128], f32)
            make_identity(nc, ident[:])
            ident_bf = constp.tile([128, 128], bf16)
            nc.vector.tensor_copy(ident_bf[:], ident[:])
            # maskM4[j, (d, i)] = 0 on j==i diagonal else 1 (4 copies)
            maskM4 = constp.tile([128, 512], bf16)
            for d in range(4):
                nc.vector.tensor_scalar(
                    maskM4[:, d * 128:(d + 1) * 128], ident_bf[:],
                    -1.0, 1.0, op0=mult, op1=add,
                )
            # zb[j, x] = 1.0 iff x == 128 (dn basis lhsT: zb[:,128-i:256-i])
            zb = constp.tile([128, 256], bf16)
            nc.vector.memset(zb[:], 0.0)
            nc.vector.memset(zb[:, 128:129], 1.0)
            ones_bf = constp.tile([1, 128], bf16)
            nc.vector.memset(ones_bf[:], 1.0)
            bias_t = constp.tile([128, 1], f32)
            nc.vector.memset(bias_t[:], EXP_BIAS)

            # v chunk tiles cmaj[c, i, j] bf16: one 128-descriptor full-rate
            # SWDGE cast DMA per chunk
            cm_tiles = []

            def load_cmaj(ck):
                cm = vp.tile([CHUNK, H, W], bf16, name="cm", tag="cm")
                nc.gpsimd.dma_start(cm[:], v_d[ck * CHUNK:(ck + 1) * CHUNK])
                cm_tiles.append(cm)

            # ---- phase 1: energies -> exp -> denominators
            with tc.tile_pool(name="qkp", bufs=1) as qkp:
                q_sb = qkp.tile([CQ, H, W], fp16)
                k_sb = qkp.tile([CQ, H, W], fp16)
                for ih in range(2):
                    sl = slice(ih * 64, ih * 64 + 64)
                    nc.gpsimd.dma_start(k_sb[:, sl, :], k_d[:, sl, :])
                    nc.gpsimd.dma_start(q_sb[:, sl, :], q_d[:, sl, :])
                load_cmaj(0)
                load_cmaj(1)

                with (
                    tc.tile_pool(name="pse", bufs=2, space="PSUM") as pse,
                    tc.tile_pool(name="psdn", bufs=1, space="PSUM") as psdn,
                ):
                    dnW_ps = [psdn.tile([128, 128], f32, name=f"dnW_ps{a}") for a in range(2)]
                    dnH_ps = [psdn.tile([128, 128], f32, name=f"dnH_ps{a}") for a in range(2)]

                    for i0 in range(0, H, 8):
                        pe = pse.tile([128, 1024], f32, name="pe_row", tag="pe")
                        for d in range(8):
                            i = i0 + d
                            nc.tensor.matmul(
                                pe[:, d * 128:(d + 1) * 128],
                                lhsT=k_sb[:, i, :], rhs=q_sb[:, i, :],
                                start=True, stop=True,
                            )
                        nc.scalar.activation(
                            att_W[:, i0 * W:(i0 + 8) * W], pe[:], Exp, bias=bias_t[:]
                        )
                        for d in range(8):
                            i = i0 + d
                            nc.tensor.matmul(
                                dnW_ps[i % 2][:], lhsT=zb[:, 128 - i:256 - i],
                                rhs=att_W[:, i * W:(i + 1) * W],
                                start=(i < 2), stop=(i >= H - 2),
                            )
                    for w0 in range(0, W, 8):
                        pe = pse.tile([128, 1024], f32, name="pe_col", tag="pe")
                        for d in range(8):
                            w = w0 + d
                            nc.tensor.matmul(
                                pe[:, d * 128:(d + 1) * 128],
                                lhsT=k_sb[:, :, w], rhs=q_sb[:, :, w],
                                start=True, stop=True,
                            )
                        nc.scalar.activation(
                            att_H[:, w0 * H:(w0 + 8) * H], pe[:], Exp, bias=bias_t[:]
                        )
                        for h2 in range(2):
                            sl = att_H[:, (w0 + 4 * h2) * H:(w0 + 4 * h2 + 4) * H]
                            nc.vector.tensor_tensor(sl, sl, maskM4[:], op=mult)
                        for d in range(8):
                            w = w0 + d
                            nc.tensor.matmul(
                                dnH_ps[w % 2][:], lhsT=zb[:, 128 - w:256 - w],
                                rhs=att_H[:, w * H:(w + 1) * H],
                                start=(w < 2), stop=(w >= H - 2),
                            )

                    # dn_iw = dnW + dnH^T ; dn_wi = dnH + dnW^T ; reciprocals
                    dnW_sb = dnp.tile([128, 128], f32)
                    nc.vector.tensor_copy(dnW_sb[:], dnW_ps[0][:])
                    nc.vector.tensor_tensor(dnW_sb[:], dnW_sb[:], dnW_ps[1][:], op=add)
                    dnH_sb = dnp.tile([128, 128], f32)
                    nc.vector.tensor_copy(dnH_sb[:], dnH_ps[0][:])
                    nc.vector.tensor_tensor(dnH_sb[:], dnH_sb[:], dnH_ps[1][:], op=add)
                    t1 = pse.tile([128, 128], f32, name="t1", tag="pe")
                    nc.tensor.transpose(t1[:], dnW_sb[:], ident[:])  # [w, i]
                    t2 = pse.tile([128, 128], f32, name="t2", tag="pe")
                    nc.tensor.transpose(t2[:], dnH_sb[:], ident[:])  # [i, w]
                    r_iw = dnp.tile([128, 128], f32)
                    nc.vector.tensor_tensor(r_iw[:], t2[:], dnW_sb[:], op=add)
                    nc.vector.reciprocal(r_iw[:], r_iw[:])
                    r_wi = dnp.tile([128, 128], f32)
                    nc.vector.tensor_tensor(r_wi[:], t1[:], dnH_sb[:], op=add)
                    nc.vector.reciprocal(r_wi[:], r_wi[:])
                    r_iw_bf = dnp.tile([128, 128], bf16)
                    nc.vector.tensor_copy(r_iw_bf[:], r_iw[:])
                    r_wi_bf = dnp.tile([128, 128], bf16)
                    nc.vector.tensor_copy(r_wi_bf[:], r_wi[:])

            # ---- phase 1b + 2: att scaling, transposed-v operands, merged PV
            with (
                tc.tile_pool(name="u2p", bufs=1) as u2p,
                tc.tile_pool(name="zp", bufs=2) as zp,
                tc.tile_pool(name="outp", bufs=3) as outp,
                tc.tile_pool(name="pst", bufs=2, space="PSUM") as pst,
            ):
                evac_ct = [0]

                def evac_t(dst, src):
                    # transpose-psum evacuations: bf16 runs 2x on DVE, so DVE
                    # takes nearly all of them (out_sb evacs go to ACT)
                    idx = evac_ct[0]
                    evac_ct[0] += 1
                    if idx % 32 == 17:
                        nc.scalar.copy(dst, src)
                    else:
                        nc.vector.tensor_copy(dst, src)

                def build_u2(cm):
                    # u2[j, w, c] = v[c0+c, j, w] (col-pass lhsT)
                    u2 = u2p.tile([128, W, CHUNK], bf16, name="u2")
                    for oct8 in range(16):
                        ps = pst.tile([128, 1024], bf16, name="ps_t", tag="pst")
                        for t in range(8):
                            x = oct8 * 8 + t
                            nc.tensor.transpose(
                                ps[:, t * 128:(t + 1) * 128], cm[:, :, x], ident_bf[:])
                        evac_t(u2[:, oct8 * 8:oct8 * 8 + 8, :].rearrange(
                            "j w c -> j (w c)"), ps[:])
                    return u2

                def build_zg(cm, i0):
                    # zg[j, d, c] = v[c0+c, i0+d, j] (row-pass lhsT, 8 rows)
                    zg = zp.tile([128, GI, CHUNK], bf16, name="zg")
                    ps = pst.tile([128, 1024], bf16, name="ps_t", tag="pst")
                    for d in range(GI):
                        nc.tensor.transpose(
                            ps[:, d * 128:(d + 1) * 128], cm[:, i0 + d, :], ident_bf[:])
                    evac_t(zg[:].rearrange("j d c -> j (d c)"), ps[:])
                    return zg

                # ---- att scaling: att *= 1/dn (pixel-wise, bcast over j).
                # 8 eighths: one sync-queue DMA refills r2[1, 4096] (32 flat
                # recip rows); 8 rank-1 matmuls broadcast it to psum; DVE
                # multiplies att (odd quads through an ACT-staged bf16 copy
                # so the mult runs in DVE fast mode). att_H eighths first:
                # every merged group's col matmuls touch all of att_H.
                # chunk-0 u2 transposes interleave to fill the PE window.
                u2 = None
                with (
                    tc.tile_pool(name="r2p", bufs=2) as r2p,
                    tc.tile_pool(name="prbp", bufs=2) as prbp,
                    tc.tile_pool(name="psr", bufs=2, space="PSUM") as psr,
                ):
                    for e in range(8):
                        r2 = r2p.tile([1, 4096], bf16, name="r2")
                        src = r_wi_bf if e < 4 else r_iw_bf
                        r0 = 32 * (e % 4)
                        deng = nc.sync if e % 2 == 0 else nc.scalar
                        deng.dma_start(r2[0:1, :], src[r0:r0 + 32, :])
                        for g in range(8):
                            pr = psr.tile([128, 512], f32, name="pr")
                            nc.tensor.matmul(
                                pr[:], lhsT=ones_bf[0:1, :],
                                rhs=r2[0:1, 512 * g:512 * g + 512],
                                start=True, stop=True,
                            )
                            if e < 4:
                                w0 = r0 + 4 * g
                                sl = att_H[:, w0 * H:(w0 + 4) * H]
                            else:
                                i0 = r0 + 4 * g
                                sl = att_W[:, i0 * W:(i0 + 4) * W]
                            if g % 2 == 0:
                                nc.vector.tensor_tensor(sl, sl, pr[:], op=mult)
                            else:
                                prb = prbp.tile([128, 512], bf16, name="prb")
                                nc.scalar.copy(prb[:], pr[:])
                                nc.vector.tensor_tensor(sl, sl, prb[:], op=mult)
                        if e == 3:
                            # att_H fully scaled; PE fills with chunk-0 u2
                            u2 = build_u2(cm_tiles[0])

                    # ---- phase 2: merged-psum PV per chunk / 8-i group
                    with tc.tile_pool(name="pgp", bufs=2, space="PSUM") as pgp:
                        zg_next = build_zg(cm_tiles[0], 0)
                        for ck in range(N_CHUNKS):
                            cm = cm_tiles[ck]
                            # chunks 0/1 prefetched in phase 1; later loads
                            # rotate into the buffer freed by chunk ck-1
                            if ck >= 1 and ck + 1 < N_CHUNKS:
                                load_cmaj(ck + 1)
                            if ck > 0:
                                u2 = build_u2(cm)

                            for g in range(NG):
                                i0 = g * GI
                                zg = zg_next
                                # pre-transpose the next group's z rows
                                if g + 1 < NG:
                                    zg_next = build_zg(cm, i0 + GI)
                                elif ck + 1 < N_CHUNKS:
                                    zg_next = build_zg(cm_tiles[ck + 1], 0)

                                pg = pgp.tile([128, GI, W], f32, name="pg")
                                # rows: one zero-region start per psum bank
                                for d in range(GI):
                                    i = i0 + d
                                    nc.tensor.matmul(
                                        pg[:, d, :], lhsT=zg[:, d, :],
                                        rhs=att_W[:, i * W:(i + 1) * W],
                                        start=(d % 4 == 0), stop=False,
                                        skip_group_check=True,
                                    )
                                # cols: strided 4-i within-bank writes
                                for w in range(W):
                                    for hb in range(2):
                                        nc.tensor.matmul(
                                            pg[:, hb * 4:hb * 4 + 4, w],
                                            lhsT=u2[:, w, :],
                                            rhs=att_H[:, w * H + i0 + hb * 4:
                                                      w * H + i0 + hb * 4 + 4],
                                            start=False,
                                            stop=(w == W - 1),
                                            skip_group_check=True,
                                        )
                                out_sb = outp.tile([128, GI, W], f32, name="out_sb")
                                nc.scalar.copy(
                                    out_sb[:].rearrange("c a b -> c (a b)"),
                                    pg[:].rearrange("c a b -> c (a b)"))
                                nc.sync.dma_start(
                                    o_d[ck * CHUNK:(ck + 1) * CHUNK, i0:i0 + GI, :],
                                    out_sb[:],
                                )

    nc.compile()
    return nc


_CACHE = {}
_LOCK = threading.Lock()


def _get_nc():
    with _LOCK:
        if "nc" not in _CACHE:
            _CACHE["nc"] = build_nc()
        return _CACHE["nc"]


def kernel(proj_query: np.ndarray, proj_key: np.ndarray, proj_value: np.ndarray,
           trace: bool = False):
    from concourse.bass_utils import run_bass_kernel_spmd

    q = np.ascontiguousarray(np.asarray(proj_query, dtype=np.float32))
    k = np.ascontiguousarray(np.asarray(proj_key, dtype=np.float32))
    v = np.ascontiguousarray(np.asarray(proj_value, dtype=np.float32))
    assert q.shape == (B, CQ, H, W) and v.shape == (B, CV, H, W)

    nc = _get_nc()
    in_maps = [{"q": q[b], "k": k[b], "v": v[b]} for b in range(B)]
    res = run_bass_kernel_spmd(nc, in_maps, core_ids=list(range(B)), trace=trace)
    out = np.stack([res.results[b]["o"] for b in range(B)], axis=0)
    if trace:
        kernel.last_exec_time_ns = res.exec_time_ns
        kernel.last_results = res
    return out


if __name__ == "__main__":
    nc = build_nc()
    print("build ok:", nc)
